# revision 1
# baseline (speedup 1.0000x reference)
"""DetectorLoss on 8 Trainium2 NeuronCores (Bass/Tile).

Strategy (data-parallel over batch, 4 images per core):
  * Host zips pred_delta_box / pred_obj / pred_cls into one [cell, 25]
    bf16 record tensor per core (pure transpose+concatenate layout
    transform: [dbox(4), obj(1), cls(20)] per (b,a,y,x) cell).
    K=ceil(Pmax/128) indirect DMAs (one contiguous 50B block per
    positive, one offset per partition) fetch ALL per-positive data -
    4 gather instructions instead of 12.  Positives are sorted by cell
    for DRAM locality.
  * The positive's class prob is selected from the gathered 20-wide
    strip by a host-built one-hot mask fused into the jk2 accumulation;
    Ln runs over the whole strip (no data-dependent extract).
  * pred_obj background: the SmoothL1 sum is even in x, so the host
    streams |pred_obj| as bf16.  Per chunk: ACT Square+accum gives
    sum x^2; DVE relu(a-1) then ACT Square+accum gives
    sum relu(|x|-1)^2.  sum sl1 = 0.5*(QS - T2).
  * The activation table is pinned to natural_log_exp_and_others
    (covers exp/ln/square/abs/relu) so exactly ONE ACT_TABLE_LOAD
    happens; tanh is computed via exp as 1 - 2/(e^2x+1).
  * gt-side constants (gt boxes, one-hot, fac*win etc.) are
    host-precomputed hd planes; per-positive math is packed
    x/y-interleaved ([128, 2K] ops; min/max fused via the negated-lo
    box encoding; gamma duplicated to pairs via 0-stride broadcast
    reads); abs/relu/square side-branches run on the otherwise-idle
    ACT engine to unload the DVE critical path.
  * Host combines per-core partial sums (weighted means).
"""
import numpy as np

B, A, C, H, W = 32, 3, 20, 160, 160
HW = H * W
M = 8            # cores
Bm = B // M      # images per core
NCELL = Bm * A * HW          # 307200 cells per core
REC = 25                     # record: dbox(4) obj(1) cls(20)
OBJ_F = NCELL // 128         # 2400 pred_obj columns per partition
NCHUNK = 2
FB = OBJ_F // NCHUNK
EPS = 1e-7
NH = 36                      # hd planes, in units of K columns
JB = 2 * NCHUNK              # first jk column
NCOLS = JB + 7               # partials: QS T2 jk1a jk1b jk2 jkq jkr jk3bq jk3br

_NC_CACHE = {}


def _pin_act_tables():
    """Mask every activation-function table set except
    natural_log_exp_and_others (covers exp/ln/square/abs/relu), so the
    table-load pass picks one set for the whole kernel -> 1 load."""
    import concourse.bacc as bacc_mod
    if getattr(bacc_mod, "_act_tables_pinned", False):
        return
    orig = bacc_mod.get_activation_tables
    KEEP = "natural_log_exp_and_others"

    def pinned(arch):
        t = orig(arch)
        if KEEP not in t:
            return t
        return {k: (v if k == KEEP else set()) for k, v in t.items()}

    bacc_mod.get_activation_tables = pinned
    bacc_mod._act_tables_pinned = True


def _build_nc(K):
    _pin_act_tables()
    import concourse.bass as bass
    import concourse.bacc as bacc
    import concourse.tile as tile
    from concourse import mybir

    f32 = mybir.dt.float32
    bf16 = mybir.dt.bfloat16
    op = mybir.AluOpType
    act = mybir.ActivationFunctionType

    nc = bacc.Bacc("TRN2", target_bir_lowering=False, debug=False)
    _orig_aeb = nc.all_engine_barrier
    _aeb_n = [0]

    def _aeb(*a, **kw):
        _aeb_n[0] += 1
        if _aeb_n[0] >= 2:
            return None
        return _orig_aeb(*a, **kw)

    nc.all_engine_barrier = _aeb
    rec_p = nc.dram_tensor("rec", [NCELL * REC, 1], bf16, kind="ExternalInput")
    pobj_p = nc.dram_tensor("pobj", [128, OBJ_F], bf16, kind="ExternalInput")
    ioffs_p = nc.dram_tensor("ioffs", [128, K], mybir.dt.int32,
                             kind="ExternalInput")
    hd_p = nc.dram_tensor("hd", [128, NH * K], f32, kind="ExternalInput")
    out_p = nc.dram_tensor("partials", [128, NCOLS], f32, kind="ExternalOutput")

    with tile.TileContext(nc) as tc, \
         tc.tile_pool(name="io", bufs=1) as io, \
         tc.tile_pool(name="wk", bufs=1) as wk, \
         tc.tile_pool(name="st", bufs=1) as st:
        ioffs = io.tile([128, K], mybir.dt.int32)
        nc.sync.dma_start(out=ioffs[:, :], in_=ioffs_p[:, :])
        hd = io.tile([128, NH * K], f32)
        nc.sync.dma_start(out=hd[:, :], in_=hd_p[:, :])
        partials = io.tile([128, NCOLS], f32)

        # ---- sparse record gathers: one 25-f32 block per positive ----
        dv = io.tile([128, K, REC], f32)
        for k in range(K):
            nc.gpsimd.indirect_dma_start(
                out=dv[:, k, :], out_offset=None, in_=rec_p[:, :],
                in_offset=bass.IndirectOffsetOnAxis(
                    ap=ioffs[:, k:k + 1], axis=0))
        dvf = dv[:, :, :]

        def dvap(lo, n):
            if n == 1:
                return bass.AP(tensor=dvf.tensor, offset=dvf.offset + lo,
                               ap=[dvf.ap[0], [REC, K]])
            return bass.AP(tensor=dvf.tensor, offset=dvf.offset + lo,
                           ap=[dvf.ap[0], [REC, K], [1, n]])

        # ---- streamed background over |pred_obj| (bf16) ----
        for c in range(NCHUNK):
            a = st.tile([128, FB], bf16, name=f"a{c}", tag=f"a{c}")
            nc.sync.dma_start(out=a[:, :], in_=pobj_p[:, c * FB:(c + 1) * FB])
            sq = st.tile([128, FB], bf16, name=f"sq{c}", tag=f"sq{c}")
            nc.scalar.activation(out=sq[:, :], in_=a[:, :], func=act.Square,
                                 accum_out=partials[:, c:c + 1])
            rm = st.tile([128, FB], bf16, name=f"rm{c}", tag=f"rm{c}")
            nc.vector.tensor_scalar(out=rm[:, :], in0=a[:, :], scalar1=-1.0,
                                    scalar2=0.0, op0=op.add, op1=op.max)
            t2o = st.tile([128, FB], bf16, name=f"t2o{c}", tag=f"t2o{c}")
            nc.scalar.activation(out=t2o[:, :], in_=rm[:, :], func=act.Square,
                                 accum_out=partials[:, NCHUNK + c:NCHUNK + c + 1])

        negone = wk.tile([128, 1], f32, name="negone", tag="negone")
        nc.vector.memset(negone[:, :], -1.0)

        # ---- per-positive math ----
        # hd plane slices (x/y interleaved pairs unless noted)
        pxy1 = hd[:, 0:2 * K]               # px+1, py+1
        ancWh = hd[:, 2 * K:4 * K]          # 0.5*W*anc
        B2 = hd[:, 4 * K:8 * K]             # [b2hi(2K) | b2lo_neg(2K)]
        gtwhe = hd[:, 8 * K:10 * K]         # gt wh + eps
        gtc2 = hd[:, 10 * K:12 * K]         # 2*gt center
        area2e = hd[:, 12 * K:13 * K]
        sqfw = hd[:, 13 * K:14 * K]         # sqrt(0.5 * 0.25*HW/n_img * win)
        win075 = hd[:, 14 * K:15 * K]       # 0.75 * win
        valid = hd[:, 15 * K:16 * K]
        onehot = hd[:, 16 * K:36 * K]       # k-major [K,20]

        tt = nc.vector.tensor_tensor
        ts = nc.vector.tensor_scalar
        stt = nc.vector.scalar_tensor_tensor
        A_ = nc.scalar.activation

        import itertools
        _cnt = itertools.count()

        def mk(n):
            nm = f"t{next(_cnt)}"
            return wk.tile([128, n * K], f32, name=nm, tag=nm)

        def ev(t, n=None):  # even (x) cols of interleaved [128,2K]
            a = t[:, :]
            return bass.AP(tensor=a.tensor, offset=a.offset, ap=[a.ap[0], [2, K]])

        def od(t):
            a = t[:, :]
            return bass.AP(tensor=a.tensor, offset=a.offset + 1,
                           ap=[a.ap[0], [2, K]])

        def g2(t, n):  # view [128, n*K] tile as (K, n) k-major groups
            a = t[:, :]
            return bass.AP(tensor=a.tensor, offset=a.offset,
                           ap=[a.ap[0], [n, K], [1, n]])

        def bc_ev(t):  # broadcast even (x) cols to interleaved [2K] reads
            a = t[:, :]
            return bass.AP(tensor=a.tensor, offset=a.offset,
                           ap=[a.ap[0], [2, K], [0, 2]])

        def bc_od(t):
            a = t[:, :]
            return bass.AP(tensor=a.tensor, offset=a.offset + 1,
                           ap=[a.ap[0], [2, K], [0, 2]])

        def i2(t):  # interleaved [2K] tile viewed as (K,2) for 3D-out ops
            a = t[:, :]
            return bass.AP(tensor=a.tensor, offset=a.offset,
                           ap=[a.ap[0], [2, K], [1, 2]])

        # tanh via exp: c1 = (px+1) - 2/(e^{2dx}+1)
        e2 = mk(2); A_(out=g2(e2, 2), in_=dvap(0, 2), func=act.Exp, scale=2.0)
        e2p = mk(2); A_(out=e2p[:, :], in_=e2[:, :], func=act.Copy, bias=1.0)
        re2p = mk(2); nc.vector.reciprocal(out=re2p[:, :], in_=e2p[:, :])
        c1 = mk(2); stt(out=c1[:, :], in0=re2p[:, :], scalar=-2.0, in1=pxy1,
                        op0=op.mult, op1=op.add)
        ex = mk(2); A_(out=g2(ex, 2), in_=dvap(2, 2), func=act.Exp)
        whh = mk(2); tt(out=whh[:, :], in0=ex[:, :], in1=ancWh, op=op.mult)
        B1 = mk(4)
        tt(out=B1[:, 0:2 * K], in0=c1[:, :], in1=whh[:, :], op=op.add)
        tt(out=B1[:, 2 * K:4 * K], in0=whh[:, :], in1=c1[:, :], op=op.subtract)
        mn4 = mk(4); tt(out=mn4[:, :], in0=B1[:, :], in1=B2, op=op.min)
        mx4 = mk(4); tt(out=mx4[:, :], in0=B1[:, :], in1=B2, op=op.max)
        whe1b = mk(2); ts(out=whe1b[:, :], in0=whh[:, :], scalar1=2.0,
                          scalar2=EPS, op0=op.mult, op1=op.add)
        # enclosing box + center delta
        cw = mk(2); tt(out=cw[:, :], in0=mx4[:, 0:2 * K], in1=mx4[:, 2 * K:4 * K],
                       op=op.add)
        df = mk(2); stt(out=df[:, :], in0=c1[:, :], scalar=-2.0, in1=gtc2,
                        op0=op.mult, op1=op.add)
        # angle branch, duplicated to interleaved [2K] via 0-stride reads
        sqs = mk(2); tt(out=sqs[:, :], in0=df[:, :], in1=df[:, :], op=op.mult)
        sig22 = mk(2); tt(out=i2(sig22), in0=bc_ev(sqs), in1=bc_od(sqs),
                          op=op.add)
        prod22 = mk(2); tt(out=i2(prod22), in0=bc_ev(df), in1=bc_od(df),
                           op=op.mult)
        aprod22 = mk(2); A_(out=aprod22[:, :], in_=prod22[:, :], func=act.Abs)
        rsig22 = mk(2); nc.vector.reciprocal(out=rsig22[:, :], in_=sig22[:, :])
        angle22 = mk(2); stt(out=angle22[:, :], in0=aprod22[:, :], scalar=2.0,
                             in1=rsig22[:, :], op0=op.mult, op1=op.mult)
        gamma22 = mk(2); ts(out=gamma22[:, :], in0=angle22[:, :], scalar1=0.25,
                            scalar2=-0.5, op0=op.mult, op1=op.add)
        # rho' = (df/cw)^2 (4x reference rho; 1/4 folded into gamma22)
        rcw = mk(2); nc.vector.reciprocal(out=rcw[:, :], in_=cw[:, :])
        srw = mk(2); tt(out=srw[:, :], in0=df[:, :], in1=rcw[:, :], op=op.mult)
        rho = mk(2); tt(out=rho[:, :], in0=srw[:, :], in1=srw[:, :], op=op.mult)
        # shape-cost args
        wd = mk(2); tt(out=wd[:, :], in0=whe1b[:, :], in1=gtwhe, op=op.subtract)
        wda = mk(2); A_(out=wda[:, :], in_=wd[:, :], func=act.Abs)
        mxw = mk(2); tt(out=mxw[:, :], in0=whe1b[:, :], in1=gtwhe, op=op.max)
        rmx = mk(2); nc.vector.reciprocal(out=rmx[:, :], in_=mxw[:, :])
        G4 = mk(4)
        tt(out=G4[:, 0:2 * K], in0=gamma22[:, :], in1=rho[:, :], op=op.mult)
        stt(out=G4[:, 2 * K:4 * K], in0=wda[:, :], scalar=-1.0, in1=rmx[:, :],
            op0=op.mult, op1=op.mult)
        # iou branch (first consumer is dif, after d1 - run during e4/sq/sh)
        it = mk(2); tt(out=it[:, :], in0=mn4[:, 0:2 * K], in1=mn4[:, 2 * K:4 * K],
                       op=op.add)
        it2 = mk(2); ts(out=it2[:, :], in0=it[:, :], scalar1=0.0, scalar2=None,
                        op0=op.max)
        inter = mk(1); tt(out=inter[:, :], in0=ev(it2), in1=od(it2), op=op.mult)
        area1 = mk(1); tt(out=area1[:, :], in0=ev(whe1b), in1=od(whe1b),
                          op=op.mult)
        u1 = mk(1); tt(out=u1[:, :], in0=area1[:, :], in1=area2e, op=op.add)
        u2 = mk(1); stt(out=u2[:, :], in0=inter[:, :], scalar=-1.0, in1=u1[:, :],
                        op0=op.mult, op1=op.add)
        ru = mk(1); nc.vector.reciprocal(out=ru[:, :], in_=u2[:, :])
        iou = mk(1); tt(out=iou[:, :], in0=inter[:, :], in1=ru[:, :], op=op.mult)
        ioum1 = mk(1); ts(out=ioum1[:, :], in0=iou[:, :], scalar1=-1.0,
                          scalar2=None, op0=op.add)
        popre = mk(1); tt(out=popre[:, :], in0=dvap(4, 1), in1=ioum1[:, :],
                          op=op.subtract)
        jk1a = mk(1); stt(out=jk1a[:, :], in0=ioum1[:, :], scalar=1.0,
                          in1=valid, op0=op.mult, op1=op.mult,
                          accum_out=partials[:, JB:JB + 1])
        e4 = mk(4); A_(out=e4[:, :], in_=G4[:, :], func=act.Exp)
        sq1 = mk(2); A_(out=sq1[:, :], in_=e4[:, 2 * K:4 * K], func=act.Square,
                        scale=-1.0, bias=1.0)
        sh = mk(2); A_(out=sh[:, :], in_=sq1[:, :], func=act.Square)
        dsub = mk(2); tt(out=dsub[:, :], in0=sh[:, :], in1=e4[:, 0:2 * K],
                         op=op.subtract)
        d1 = mk(1); tt(out=d1[:, :], in0=ev(dsub), in1=od(dsub), op=op.add)
        # dif = po - siou = (po - iou + 1) + 0.5*d1; sl1 via squares with
        # sqrt(0.5*fac*win) folded into both branches.  Critical chain
        # dif->ad->rmv->rmw->jkr emitted contiguously; independent accums
        # (jk1b, difw/jkq) trail it in queue order.
        dif = mk(1); stt(out=dif[:, :], in0=d1[:, :], scalar=0.5,
                         in1=popre[:, :], op0=op.mult, op1=op.add)
        ad = mk(1); stt(out=ad[:, :], in0=dif[:, :], scalar=-1.0,
                        in1=dif[:, :], op0=op.mult, op1=op.max)
        rmv = mk(1); ts(out=rmv[:, :], in0=ad[:, :], scalar1=-1.0, scalar2=0.0,
                        op0=op.add, op1=op.max)
        rmw = mk(1); tt(out=rmw[:, :], in0=rmv[:, :], in1=sqfw, op=op.mult)
        jkr = mk(1); stt(out=jkr[:, :], in0=rmw[:, :], scalar=1.0,
                         in1=rmw[:, :], op0=op.mult, op1=op.mult,
                         accum_out=partials[:, JB + 4:JB + 5])
        difw = mk(1); tt(out=difw[:, :], in0=dif[:, :], in1=sqfw, op=op.mult)
        jkq = mk(1); stt(out=jkq[:, :], in0=difw[:, :], scalar=1.0,
                         in1=difw[:, :], op0=op.mult, op1=op.mult,
                         accum_out=partials[:, JB + 3:JB + 4])
        jk1b = mk(1); stt(out=jk1b[:, :], in0=d1[:, :], scalar=-0.5,
                          in1=valid, op0=op.mult, op1=op.mult,
                          accum_out=partials[:, JB + 1:JB + 2])
        # cls loss: ln over the whole gathered strip, one-hot select fused
        lnp = mk(20); A_(out=g2(lnp, 20), in_=dvap(5, 20), func=act.Ln)
        jk2 = mk(20); stt(out=jk2[:, :], in0=lnp[:, :], scalar=-1.0, in1=onehot,
                          op0=op.mult, op1=op.mult,
                          accum_out=partials[:, JB + 2:JB + 3])
        poq = mk(1); A_(out=poq[:, :], in_=dvap(4, 1), func=act.Square)
        jk3bq = mk(1); stt(out=jk3bq[:, :], in0=poq[:, :], scalar=0.5,
                           in1=win075, op0=op.mult, op1=op.mult,
                           accum_out=partials[:, JB + 5:JB + 6])
        poa = mk(1); A_(out=poa[:, :], in_=dvap(4, 1), func=act.Abs)
        por = mk(1); A_(out=por[:, :], in_=poa[:, :], func=act.Relu,
                        bias=negone[:, 0:1])
        por2 = mk(1); A_(out=por2[:, :], in_=por[:, :], func=act.Square)
        jk3br = mk(1); stt(out=jk3br[:, :], in0=por2[:, :], scalar=0.5,
                           in1=win075, op0=op.mult, op1=op.mult,
                           accum_out=partials[:, JB + 6:JB + 7])

        nc.sync.dma_start(out=out_p[:, :], in_=partials[:, :])

    return nc


def _get_nc(K, finalized=True):
    key = (K, finalized)
    if key not in _NC_CACHE:
        nc = _build_nc(K)
        if finalized:
            nc.finalize()
        else:
            nc.compile()
        _NC_CACHE[key] = nc
    return _NC_CACHE[key]


def _pack(vals, K, fill, dtype=np.float32):
    """lane j = i*128 + p  ->  tile[p, i]."""
    out = np.full((K, 128), fill, dtype)
    out.reshape(-1)[:len(vals)] = vals
    return out.T


def _pack2(vx, vy, K, fill):
    """x/y pair -> interleaved cols (i*2, i*2+1) for lane j = i*128+p."""
    out = np.full((K, 2, 128), fill, np.float32)
    n = len(vx)
    flat = out.reshape(K * 2, 128)
    j = np.arange(n)
    flat[(j // 128) * 2, j % 128] = vx
    flat[(j // 128) * 2 + 1, j % 128] = vy
    return flat.T


def _packoh(cj, K):
    """one-hot class mask, k-major [K,20] cols for lane j."""
    out = np.zeros((K * 20, 128), np.float32)
    j = np.arange(len(cj))
    out[(j // 128) * 20 + cj, j % 128] = 1.0
    return out.T


def host_prep(pred_obj, pred_delta_box, pred_cls, gt_box, gt_cls,
              p_batch_idx, p_x_idx, p_y_idx, p_anchor_idx, anchors):
    """Shard inputs across cores; build record tensor + index/const planes."""
    import ml_dtypes
    f32 = np.float32
    pred_obj = np.asarray(pred_obj, f32)
    pdb = np.asarray(pred_delta_box, f32)
    pcls = np.asarray(pred_cls, f32)
    gtb = np.asarray(gt_box, f32)
    gcls = np.asarray(gt_cls, np.int64)
    p_b = np.asarray(p_batch_idx, np.int64)
    p_x = np.asarray(p_x_idx, np.int64)
    p_y = np.asarray(p_y_idx, np.int64)
    p_a = np.asarray(p_anchor_idx, np.int64)
    anchors = np.asarray(anchors, f32)
    P = len(p_b)

    n_img = np.bincount(p_b, minlength=B)
    # duplicate (b,y,x,a) cells: last occurrence wins (matches XLA scatter)
    cell_g = ((p_b * H + p_y) * W + p_x) * A + p_a
    win = np.zeros(P, f32)
    _, ridx = np.unique(cell_g[::-1], return_index=True)
    win[P - 1 - ridx] = 1.0

    core_of = p_b // Bm
    counts = np.bincount(core_of, minlength=M)
    K = max(1, -(-int(counts.max()) // 128))

    in_maps = []
    for m in range(M):
        sel = np.nonzero(core_of == m)[0]
        bl = p_b[sel] - m * Bm
        aj = p_a[sel]
        cell = (bl * A + aj) * HW + p_y[sel] * W + p_x[sel]
        sel = sel[np.argsort(cell, kind="stable")]
        bl = p_b[sel] - m * Bm
        xj, yj, aj, cj = p_x[sel], p_y[sel], p_a[sel], gcls[sel]
        cell = (bl * A + aj) * HW + yj * W + xj
        ioffs = _pack((cell * REC).astype(np.int32), K, 0, np.int32)

        gw = gtb[sel, 2] + EPS
        gh = gtb[sel, 3] + EPS
        gx = gtb[sel, 0]
        gy = gtb[sel, 1]
        anc = anchors[aj]
        hd = np.concatenate([
            _pack2(xj + 1.0, yj + 1.0, K, 1.0),
            _pack2(0.5 * W * anc[:, 0], 0.5 * H * anc[:, 1], K, 0.1),
            _pack2(gx + 0.5 * gw, gy + 0.5 * gh, K, 1.0),
            _pack2(0.5 * gw - gx, 0.5 * gh - gy, K, 0.5),
            _pack2(gw, gh, K, 0.5),
            _pack2(2.0 * gx, 2.0 * gy, K, 1.0),
            _pack(gw * gh + EPS, K, 0.3),
            _pack(np.sqrt(0.125 * HW / n_img[p_b[sel]] * win[sel]), K, 0.0),
            _pack(0.75 * win[sel], K, 0.0),
            _pack(np.ones(len(sel), f32), K, 0.0),
            _packoh(cj, K),
        ], axis=1)

        sl = slice(m * Bm, (m + 1) * Bm)
        rec = np.empty((Bm, A, H, W, REC), ml_dtypes.bfloat16)
        rec[..., 0:4] = pdb[sl].transpose(0, 1, 3, 4, 2)
        rec[..., 4] = pred_obj[sl]
        rec[..., 5:] = pcls[sl].transpose(0, 1, 3, 4, 2)

        pobj = np.abs(pred_obj[sl]).reshape(128, OBJ_F).astype(ml_dtypes.bfloat16)

        in_maps.append({
            "rec": rec.reshape(NCELL * REC, 1),
            "pobj": pobj,
            "ioffs": np.ascontiguousarray(ioffs),
            "hd": np.ascontiguousarray(hd),
        })
    return in_maps, K, P


def combine(partials_list, P):
    """Host reduction of per-core [128, NCOLS] partial sums."""
    QS = T2 = sv = jk2 = jk3a = jk3b = 0.0
    for pt in partials_list:
        pt = np.asarray(pt, np.float64)
        QS += pt[:, 0:NCHUNK].sum()
        T2 += pt[:, NCHUNK:2 * NCHUNK].sum()
        jb = 2 * NCHUNK
        sv += pt[:, jb].sum() + pt[:, jb + 1].sum()
        jk2 += pt[:, jb + 2].sum()
        jk3a += pt[:, jb + 3].sum() - pt[:, jb + 4].sum()
        jk3b += pt[:, jb + 5].sum() - pt[:, jb + 6].sum()
    iou_loss = (P - sv) / P
    cls_loss = jk2 / P
    obj_loss = (0.375 * (QS - T2) + jk3a - jk3b) / (B * A * H * W)
    tot_loss = iou_loss + 4 * obj_loss + 2 * cls_loss
    return (np.float32(iou_loss), np.float32(obj_loss),
            np.float32(cls_loss), np.float32(tot_loss))


def kernel(pred_obj, pred_delta_box, pred_cls, gt_box, gt_cls,
           p_batch_idx, p_x_idx, p_y_idx, p_anchor_idx, anchors):
    from concourse.bass_utils import run_bass_kernel_spmd
    in_maps, K, P = host_prep(pred_obj, pred_delta_box, pred_cls, gt_box,
                              gt_cls, p_batch_idx, p_x_idx, p_y_idx,
                              p_anchor_idx, anchors)
    nc = _get_nc(K)
    res = run_bass_kernel_spmd(nc, in_maps, list(range(M))).results
    return combine([r["partials"] for r in res], P)

